# revision 5
# baseline (speedup 1.0000x reference)
"""Trainium2 Bass kernel for LyapunovSDELayer.

Reference computes, per batch element b with lam0 = current_lyapunov[b, 0]:
    path[b, 0] = lam0
    path[b, t] = clip(path[b, t-1] + KAPPA*(THETA - path[b, t-1]), 0, 1)

The step map is affine: lam -> 0.5*lam + 0.15, and for lam0 in [0, 1) the
iterates stay inside [0.15, 0.65] so the clip never binds.  Hence

    path[b, t] = THETA + 0.5**t * (lam0 - THETA)

0.5**t is a power of two so w_t * d is exact in fp32 and
fl(THETA + w_t*d) matches the reference fp32 scan to ~1 ulp; for
t >= 32 the value is exactly fl32(THETA) (the scan converges by t=26).

The kernel is a pure HBM-store-bandwidth problem: 16 MB/core, and with
all 8 cores streaming the chip HBM (~2.9 TB/s) is the roofline, so the
only wins are starting the store stream early, keeping all 16 SDMA
engines demanding until the end, and avoiding stalls:

  * the output is split into a `heads` region ([rows, 32], computed) and
    a `tails` region ([rows, 224], the constant fl32(THETA)); the host
    reassembles columns.  Tail stores read one small constant SBUF tile
    (read-only, reused by every store) with NO input dependency, so the
    stream starts right after the fixed NEFF preamble and has no
    write-after-read rotation stalls.
  * heads are computed in two whole-tile passes (DVE broadcast
    tensor_tensor for w_t*d, then one ACT activation for +THETA) and
    stored via the Activation HWDGE queue, so they interleave with the
    tail stream mid-flight (engines round-robin between the two rings).
  * all DRAM store regions are padded so per-partition runs never
    collapse into one contiguous block: a collapsed AP takes the slow
    8-engine "spray" path (~12 B/ns/engine vs 26.5 for strided stores).
"""

import sys
import types

import numpy as np

import concourse.bacc as bacc
import concourse.mybir as mybir
from concourse.tile import TileContext
from concourse.bass_utils import run_bass_kernel_spmd

# If BASS_TRACE is set in the environment, run_bass_kernel_spmd imports
# antenv.axon_hooks, which this image lacks — register a no-op stub so
# that path degrades to "no trace" instead of crashing.
try:
    import antenv.axon_hooks  # noqa: F401
except ImportError:
    try:
        import antenv

        _stub = types.ModuleType("antenv.axon_hooks")
        _stub.get_axon_ntff_profile_hook = lambda: None
        _stub.set_axon_ntff_profile_hook = lambda h: None
        sys.modules["antenv.axon_hooks"] = _stub
        antenv.axon_hooks = _stub
    except Exception:
        pass

THETA = 0.3
KAPPA = 0.5
N_CORES = 8
P = 128

# tail-store row schedule (rows per uniform all-partition store) and the
# progressive constant-tile fill row boundaries that gate them
SCHED = [1, 2, 4, 5, 8, 16, 16, 16, 16, 16, 16, 12]
FILLS = [1, 4, 8, 16]
PAD = 16  # free-dim padding (elements) keeping DRAM APs partition-strided

_NC_CACHE = {}

# test harness hook: set by test.py to capture BassKernelResults
LAST_RESULTS = None
TRACE = False


def _build(bpc: int, H: int):
    T = min(32, H)
    TL = H - T
    f32 = mybir.dt.float32
    R = bpc // P
    assert R * P == bpc
    assert sum(SCHED) == R
    CG = FILLS[-1]
    assert max(SCHED) <= CG

    nc = bacc.Bacc()
    wl = nc.dram_tensor("wl", [P, T + R], f32, kind="ExternalInput")
    heads = nc.dram_tensor("heads", [P, R * T + PAD], f32, kind="ExternalOutput")
    tails = nc.dram_tensor("tails", [P, R * TL + PAD], f32, kind="ExternalOutput")

    with TileContext(nc) as tc:
        with tc.tile_pool(name="work", bufs=1) as pool:
            wl_sb = pool.tile([P, T + R], f32)
            ct = pool.tile([P, CG * TL], f32)
            prod = pool.tile([P, R * T], f32)
            ht = pool.tile([P, R * T], f32)

            # DVE: progressive constant-tile fill (emitted first so the
            # stores below only RAW-depend on the covering memset)
            f0 = 0
            for f1 in FILLS:
                nc.vector.memset(ct[:, f0 * TL : f1 * TL], THETA)
                f0 = f1

            # SP queue: first tail store goes out the moment the first
            # memset lands, input load right behind it (its ~2 us latency
            # hides under the tail stream; only the heads need it).
            # Emission order IS program order for Tile's dependency
            # tracking: the input load must be emitted before the
            # tensor_tensor that reads wl_sb, else the load is treated as
            # a WAR hazard and waits for the compute.
            nc.sync.dma_start(
                out=tails[:, : SCHED[0] * TL], in_=ct[:, : SCHED[0] * TL]
            )
            nc.sync.dma_start(out=wl_sb, in_=wl[:, :])
            r0 = SCHED[0]
            for g in SCHED[1:]:
                nc.sync.dma_start(
                    out=tails[:, r0 * TL : (r0 + g) * TL], in_=ct[:, : g * TL]
                )
                r0 += g
            assert r0 == R

            # DVE: head product prod = w_t * d (waits on the input load)
            wt = wl_sb[:, :T]
            d = wl_sb[:, T : T + R]
            d3 = d.rearrange("p (r one) -> p r one", one=1).broadcast_to((P, R, T))
            w3 = wt.rearrange("p (one t) -> p one t", one=1).broadcast_to((P, R, T))
            p3 = prod.rearrange("p (r t) -> p r t", t=T)
            nc.vector.tensor_tensor(out=p3, in0=d3, in1=w3, op=mybir.AluOpType.mult)

            # ACT: +THETA pass, then the head store on the ACT HWDGE queue
            nc.scalar.activation(
                out=ht,
                in_=prod,
                func=mybir.ActivationFunctionType.Copy,
                bias=THETA,
                scale=1.0,
            )
            nc.scalar.dma_start(out=heads[:, : R * T], in_=ht[:, :])
    nc.finalize()
    return nc


def kernel(current_lyapunov: np.ndarray, horizon) -> np.ndarray:
    global LAST_RESULTS
    lam0 = np.ascontiguousarray(np.asarray(current_lyapunov, np.float32)).reshape(-1)
    H = int(horizon)
    B = lam0.shape[0]
    assert B % (N_CORES * P) == 0
    bpc = B // N_CORES
    R = bpc // P
    T = min(32, H)
    TL = H - T

    key = (bpc, H)
    if key not in _NC_CACHE:
        _NC_CACHE[key] = _build(bpc, H)
    nc = _NC_CACHE[key]

    # w_t = 0.5**t exact powers of two; d = lam0 - THETA (numpy fp32 sub
    # == device fp32 sub, bit-identical)
    w = (0.5 ** np.arange(T, dtype=np.float64)).astype(np.float32)
    d_host = (lam0 - np.float32(THETA)).astype(np.float32)
    in_maps = []
    for c in range(N_CORES):
        wlc = np.empty((P, T + R), np.float32)
        wlc[:, :T] = w
        wlc[:, T:] = d_host[c * bpc : (c + 1) * bpc].reshape(P, R)
        in_maps.append({"wl": wlc})

    res = run_bass_kernel_spmd(
        nc,
        in_maps,
        core_ids=list(range(N_CORES)),
        trace=TRACE,
    )
    LAST_RESULTS = res

    out = np.empty((B, H), np.float32)
    for c in range(N_CORES):
        o = out[c * bpc : (c + 1) * bpc]
        o[:, :T] = res.results[c]["heads"][:, : R * T].reshape(bpc, T)
        o[:, T:] = res.results[c]["tails"][:, : R * TL].reshape(bpc, TL)
    return out


# revision 6
# speedup vs baseline: 1.0185x; 1.0185x over previous
"""Trainium2 Bass kernel for LyapunovSDELayer.

Reference computes, per batch element b with lam0 = current_lyapunov[b, 0]:
    path[b, 0] = lam0
    path[b, t] = clip(path[b, t-1] + KAPPA*(THETA - path[b, t-1]), 0, 1)

The step map is affine: lam -> 0.5*lam + 0.15, and for lam0 in [0, 1) the
iterates stay inside [0.15, 0.65] so the clip never binds.  Hence

    path[b, t] = THETA + 0.5**t * (lam0 - THETA)

0.5**t is a power of two so w_t * d is exact in fp32 and
fl(THETA + w_t*d) matches the reference fp32 scan to ~1 ulp; for
t >= 32 the value is exactly fl32(THETA) (the scan converges by t=26).

The kernel is a pure HBM-store-bandwidth problem (16 MB/core x 8 cores
vs ~2.9 TB/s chip HBM).  Structure:

  * the output is split into a `heads` region ([rows, 32], computed) and
    a `tails` region ([rows, 224], the constant fl32(THETA)); the host
    reassembles columns.  Tail stores read one small constant SBUF tile
    (read-only, reused by every store) with NO input dependency, so the
    stream starts right after the fixed NEFF preamble and has no
    write-after-read rotation stalls.
  * HWDGE descriptors are dealt to the 16 SDMA engines in blocks of 8
    by descriptor index, and engine 15 is reproducibly ~20% slower under
    load (21 vs 26 B/ns).  Partitions 120-127 (whose descriptors always
    land on engine 15) therefore carry 98 rows instead of 130; the
    balancing "extra" stores cover partitions 0:120 only, whose 120
    descriptors engine 15 never serves.  This equalizes engine finish
    times (~41 us) instead of letting engine 15 trail by ~10 us.
  * heads are computed in two whole-tile passes (DVE broadcast
    tensor_tensor for w_t*d, one ACT activation for +THETA) and stored
    via the Activation HWDGE queue so they interleave with the tail
    stream mid-flight.
  * emission order IS program order for Tile's dependency tracking: the
    input load is emitted before the tensor_tensor that reads it, and
    all constant-tile memsets are emitted before any store that reads
    the tile, so the only waits are genuine RAW edges.
  * all DRAM store regions are padded so per-partition runs never
    collapse into one contiguous block: a collapsed AP takes the slow
    8-engine "spray" path (~12 B/ns/engine vs 26.5 for strided stores).
"""

import sys
import types

import numpy as np

import concourse.bacc as bacc
import concourse.mybir as mybir
from concourse.tile import TileContext
from concourse.bass_utils import run_bass_kernel_spmd

# If BASS_TRACE is set in the environment, run_bass_kernel_spmd imports
# antenv.axon_hooks, which this image lacks — register a no-op stub so
# that path degrades to "no trace" instead of crashing.
try:
    import antenv.axon_hooks  # noqa: F401
except ImportError:
    try:
        import antenv

        _stub = types.ModuleType("antenv.axon_hooks")
        _stub.get_axon_ntff_profile_hook = lambda: None
        _stub.set_axon_ntff_profile_hook = lambda h: None
        sys.modules["antenv.axon_hooks"] = _stub
        antenv.axon_hooks = _stub
    except Exception:
        pass

THETA = 0.3
KAPPA = 0.5
N_CORES = 8
P = 128

# rows per partition: partitions 0..119 vs engine-15 partitions 120..127
R_F = 130
R_S = 98
N_SLOW = 8
N_FAST = P - N_SLOW
# uniform-row store schedule (all 128 partitions, R_S rows total) and
# extra-row schedule (partitions 0:N_FAST, R_F - R_S rows total)
UNIFORM_SCHED = [1, 2, 4, 5, 8, 16, 16, 16, 16, 14]
EXTRA_SCHED = [16, 16]
FILLS = [1, 4, 8, 16]  # progressive constant-tile fill boundaries (rows)
PAD = 16  # free-dim padding (elements) keeping DRAM APs partition-strided

_NC_CACHE = {}

# test harness hook: set by test.py to capture BassKernelResults
LAST_RESULTS = None
TRACE = False


def _build(bpc: int, H: int):
    T = min(32, H)
    TL = H - T
    f32 = mybir.dt.float32
    assert bpc == N_FAST * R_F + N_SLOW * R_S
    assert sum(UNIFORM_SCHED) == R_S and sum(EXTRA_SCHED) == R_F - R_S
    CG = FILLS[-1]
    assert max(UNIFORM_SCHED + EXTRA_SCHED) <= CG

    nc = bacc.Bacc()
    wl = nc.dram_tensor("wl", [P, T + R_F], f32, kind="ExternalInput")
    heads = nc.dram_tensor("heads", [P, R_F * T + PAD], f32, kind="ExternalOutput")
    tails = nc.dram_tensor("tails", [P, R_F * TL + PAD], f32, kind="ExternalOutput")

    with TileContext(nc) as tc:
        with tc.tile_pool(name="work", bufs=1) as pool:
            wl_sb = pool.tile([P, T + R_F], f32)
            ct = pool.tile([P, CG * TL], f32)
            prod = pool.tile([P, R_F * T], f32)
            ht = pool.tile([P, R_F * T], f32)

            # DVE: progressive constant-tile fill (all emitted before any
            # store reads the tile, so stores carry only RAW edges)
            f0 = 0
            for f1 in FILLS:
                nc.vector.memset(ct[:, f0 * TL : f1 * TL], THETA)
                f0 = f1

            # SP queue: first tail store the moment the first memset
            # lands, input load right behind it (its ~2 us latency hides
            # under the tail stream; only the heads depend on it)
            g0 = UNIFORM_SCHED[0]
            nc.sync.dma_start(out=tails[:, : g0 * TL], in_=ct[:, : g0 * TL])
            nc.sync.dma_start(out=wl_sb, in_=wl[:, :])
            r0 = g0
            for g in UNIFORM_SCHED[1:]:
                nc.sync.dma_start(
                    out=tails[:, r0 * TL : (r0 + g) * TL], in_=ct[:, : g * TL]
                )
                r0 += g
            assert r0 == R_S
            for g in EXTRA_SCHED:
                nc.sync.dma_start(
                    out=tails[:N_FAST, r0 * TL : (r0 + g) * TL],
                    in_=ct[:N_FAST, : g * TL],
                )
                r0 += g
            assert r0 == R_F

            # DVE: head product prod = w_t * d (RAW on the input load)
            wt = wl_sb[:, :T]
            d = wl_sb[:, T : T + R_F]
            d3 = d.rearrange("p (r one) -> p r one", one=1).broadcast_to((P, R_F, T))
            w3 = wt.rearrange("p (one t) -> p one t", one=1).broadcast_to((P, R_F, T))
            p3 = prod.rearrange("p (r t) -> p r t", t=T)
            nc.vector.tensor_tensor(out=p3, in0=d3, in1=w3, op=mybir.AluOpType.mult)

            # ACT: +THETA pass, then the head stores on the ACT HWDGE
            # queue (engines round-robin between the two rings, so heads
            # interleave into the tail stream mid-flight)
            nc.scalar.activation(
                out=ht,
                in_=prod,
                func=mybir.ActivationFunctionType.Copy,
                bias=THETA,
                scale=1.0,
            )
            nc.scalar.dma_start(out=heads[:, : R_S * T], in_=ht[:, : R_S * T])
            nc.scalar.dma_start(
                out=heads[:N_FAST, R_S * T : R_F * T],
                in_=ht[:N_FAST, R_S * T : R_F * T],
            )
    nc.finalize()
    return nc


def kernel(current_lyapunov: np.ndarray, horizon) -> np.ndarray:
    global LAST_RESULTS
    lam0 = np.ascontiguousarray(np.asarray(current_lyapunov, np.float32)).reshape(-1)
    H = int(horizon)
    B = lam0.shape[0]
    assert B % N_CORES == 0
    bpc = B // N_CORES
    T = min(32, H)
    TL = H - T

    key = (bpc, H)
    if key not in _NC_CACHE:
        _NC_CACHE[key] = _build(bpc, H)
    nc = _NC_CACHE[key]

    # w_t = 0.5**t exact powers of two; d = lam0 - THETA (numpy fp32 sub
    # == device fp32 sub, bit-identical)
    w = (0.5 ** np.arange(T, dtype=np.float64)).astype(np.float32)
    d_host = (lam0 - np.float32(THETA)).astype(np.float32)
    nf_rows = N_FAST * R_F
    in_maps = []
    for c in range(N_CORES):
        dc = d_host[c * bpc : (c + 1) * bpc]
        wlc = np.zeros((P, T + R_F), np.float32)
        wlc[:, :T] = w
        wlc[:N_FAST, T : T + R_F] = dc[:nf_rows].reshape(N_FAST, R_F)
        wlc[N_FAST:, T : T + R_S] = dc[nf_rows:].reshape(N_SLOW, R_S)
        in_maps.append({"wl": wlc})

    res = run_bass_kernel_spmd(
        nc,
        in_maps,
        core_ids=list(range(N_CORES)),
        trace=TRACE,
    )
    LAST_RESULTS = res

    out = np.empty((B, H), np.float32)
    for c in range(N_CORES):
        hd = res.results[c]["heads"]
        tl = res.results[c]["tails"]
        o = out[c * bpc : (c + 1) * bpc]
        o[:nf_rows, :T] = hd[:N_FAST, : R_F * T].reshape(nf_rows, T)
        o[nf_rows:, :T] = hd[N_FAST:, : R_S * T].reshape(N_SLOW * R_S, T)
        o[:nf_rows, T:] = tl[:N_FAST, : R_F * TL].reshape(nf_rows, TL)
        o[nf_rows:, T:] = tl[N_FAST:, : R_S * TL].reshape(N_SLOW * R_S, TL)
    return out


# revision 7
# speedup vs baseline: 1.0210x; 1.0025x over previous
"""Trainium2 Bass kernel for LyapunovSDELayer.

Reference computes, per batch element b with lam0 = current_lyapunov[b, 0]:
    path[b, 0] = lam0
    path[b, t] = clip(path[b, t-1] + KAPPA*(THETA - path[b, t-1]), 0, 1)

The step map is affine: lam -> 0.5*lam + 0.15, and for lam0 in [0, 1) the
iterates stay inside [0.15, 0.65] so the clip never binds.  Hence

    path[b, t] = THETA + 0.5**t * (lam0 - THETA)

0.5**t is a power of two so w_t * d is exact in fp32 and
fl(THETA + w_t*d) matches the reference fp32 scan to ~1 ulp; for
t >= 32 the value is exactly fl32(THETA) (the scan converges by t=26).

The kernel is a pure HBM-store-bandwidth problem (16 MB/core x 8 cores
vs ~2.9 TB/s chip HBM).  Structure:

  * the output is split into a `heads` region ([rows, 32], computed) and
    a `tails` region ([rows, 224], the constant fl32(THETA)); the host
    reassembles columns.  Tail stores read one small constant SBUF tile
    (read-only, reused by every store) with NO input dependency, so the
    stream starts right after the fixed NEFF preamble and has no
    write-after-read rotation stalls.
  * HWDGE descriptors are dealt to the 16 SDMA engines in blocks of 8
    by descriptor index, and engine 15 is reproducibly ~20% slower under
    load (21 vs 26 B/ns).  Partitions 120-127 (whose descriptors always
    land on engine 15) therefore carry 98 rows instead of 130; the
    balancing "extra" stores cover partitions 0:120 only, whose 120
    descriptors engine 15 never serves.  This equalizes engine finish
    times (~41 us) instead of letting engine 15 trail by ~10 us.
  * heads are computed in two whole-tile passes (DVE broadcast
    tensor_tensor for w_t*d, one ACT activation for +THETA) and stored
    via the Activation HWDGE queue so they interleave with the tail
    stream mid-flight.
  * emission order IS program order for Tile's dependency tracking: the
    input load is emitted before the tensor_tensor that reads it, and
    all constant-tile memsets are emitted before any store that reads
    the tile, so the only waits are genuine RAW edges.
  * all DRAM store regions are padded so per-partition runs never
    collapse into one contiguous block: a collapsed AP takes the slow
    8-engine "spray" path (~12 B/ns/engine vs 26.5 for strided stores).
"""

import sys
import types

import numpy as np

import concourse.bacc as bacc
import concourse.mybir as mybir
from concourse.tile import TileContext
from concourse.bass_utils import run_bass_kernel_spmd

# If BASS_TRACE is set in the environment, run_bass_kernel_spmd imports
# antenv.axon_hooks, which this image lacks — register a no-op stub so
# that path degrades to "no trace" instead of crashing.
try:
    import antenv.axon_hooks  # noqa: F401
except ImportError:
    try:
        import antenv

        _stub = types.ModuleType("antenv.axon_hooks")
        _stub.get_axon_ntff_profile_hook = lambda: None
        _stub.set_axon_ntff_profile_hook = lambda h: None
        sys.modules["antenv.axon_hooks"] = _stub
        antenv.axon_hooks = _stub
    except Exception:
        pass

THETA = 0.3
KAPPA = 0.5
N_CORES = 8
P = 128

# rows per partition: partitions 0..119 vs engine-15 partitions 120..127
R_F = 130
R_S = 98
N_SLOW = 8
N_FAST = P - N_SLOW
# uniform-row store schedule (all 128 partitions, R_S rows total) and
# extra-row schedule (partitions 0:N_FAST, R_F - R_S rows total)
UNIFORM_SCHED = [1, 2, 4, 5, 8, 16, 16, 16, 16, 14]
EXTRA_SCHED = [16, 16]
FILLS = [1, 4, 8, 16]  # progressive constant-tile fill boundaries (rows)
PAD = 16  # free-dim padding (elements) keeping DRAM APs partition-strided

_NC_CACHE = {}

# test harness hook: set by test.py to capture BassKernelResults
LAST_RESULTS = None
TRACE = False


def _build(bpc: int, H: int):
    T = min(32, H)
    TL = H - T
    f32 = mybir.dt.float32
    assert bpc == N_FAST * R_F + N_SLOW * R_S
    assert sum(UNIFORM_SCHED) == R_S and sum(EXTRA_SCHED) == R_F - R_S
    CG = FILLS[-1]
    assert max(UNIFORM_SCHED + EXTRA_SCHED) <= CG

    nc = bacc.Bacc()
    wl = nc.dram_tensor("wl", [P, T + R_F], f32, kind="ExternalInput")
    heads = nc.dram_tensor("heads", [P, R_F * T + PAD], f32, kind="ExternalOutput")
    tails = nc.dram_tensor("tails", [P, R_F * TL + PAD], f32, kind="ExternalOutput")

    with TileContext(nc) as tc:
        with tc.tile_pool(name="work", bufs=1) as pool:
            wl_sb = pool.tile([P, T + R_F], f32)
            ct = pool.tile([P, CG * TL], f32)
            prod = pool.tile([P, R_F * T], f32)
            ht = pool.tile([P, R_F * T], f32)

            # DVE: progressive constant-tile fill (all emitted before any
            # store reads the tile, so stores carry only RAW edges)
            f0 = 0
            for f1 in FILLS:
                nc.vector.memset(ct[:, f0 * TL : f1 * TL], THETA)
                f0 = f1

            # SP queue: first tail store the moment the first memset
            # lands, input load right behind it (its ~2 us latency hides
            # under the tail stream; only the heads depend on it)
            g0 = UNIFORM_SCHED[0]
            nc.sync.dma_start(out=tails[:, : g0 * TL], in_=ct[:, : g0 * TL])
            nc.sync.dma_start(out=wl_sb, in_=wl[:, :])
            r0 = g0
            for g in UNIFORM_SCHED[1:]:
                nc.sync.dma_start(
                    out=tails[:, r0 * TL : (r0 + g) * TL], in_=ct[:, : g * TL]
                )
                r0 += g
            assert r0 == R_S
            for g in EXTRA_SCHED:
                nc.sync.dma_start(
                    out=tails[:N_FAST, r0 * TL : (r0 + g) * TL],
                    in_=ct[:N_FAST, : g * TL],
                )
                r0 += g
            assert r0 == R_F

            # DVE: head product prod = w_t * d (RAW on the input load)
            wt = wl_sb[:, :T]
            d = wl_sb[:, T : T + R_F]
            d3 = d.rearrange("p (r one) -> p r one", one=1).broadcast_to((P, R_F, T))
            w3 = wt.rearrange("p (one t) -> p one t", one=1).broadcast_to((P, R_F, T))
            p3 = prod.rearrange("p (r t) -> p r t", t=T)
            nc.vector.tensor_tensor(out=p3, in0=d3, in1=w3, op=mybir.AluOpType.mult)

            # ACT: +THETA pass, then the head stores on the ACT HWDGE
            # queue (engines round-robin between the two rings, so heads
            # interleave into the tail stream mid-flight)
            nc.scalar.activation(
                out=ht,
                in_=prod,
                func=mybir.ActivationFunctionType.Copy,
                bias=THETA,
                scale=1.0,
            )
            nc.scalar.dma_start(out=heads[:, : R_S * T], in_=ht[:, : R_S * T])
            nc.scalar.dma_start(
                out=heads[:N_FAST, R_S * T : R_F * T],
                in_=ht[:N_FAST, R_S * T : R_F * T],
            )
    nc.finalize()
    return nc


def kernel(current_lyapunov: np.ndarray, horizon) -> np.ndarray:
    global LAST_RESULTS
    lam0 = np.ascontiguousarray(np.asarray(current_lyapunov, np.float32)).reshape(-1)
    H = int(horizon)
    B = lam0.shape[0]
    assert B % N_CORES == 0
    bpc = B // N_CORES
    T = min(32, H)
    TL = H - T

    key = (bpc, H)
    if key not in _NC_CACHE:
        _NC_CACHE[key] = _build(bpc, H)
    nc = _NC_CACHE[key]

    # w_t = 0.5**t exact powers of two; d = lam0 - THETA (numpy fp32 sub
    # == device fp32 sub, bit-identical)
    w = (0.5 ** np.arange(T, dtype=np.float64)).astype(np.float32)
    d_host = (lam0 - np.float32(THETA)).astype(np.float32)
    nf_rows = N_FAST * R_F
    in_maps = []
    for c in range(N_CORES):
        dc = d_host[c * bpc : (c + 1) * bpc]
        wlc = np.zeros((P, T + R_F), np.float32)
        wlc[:, :T] = w
        wlc[:N_FAST, T : T + R_F] = dc[:nf_rows].reshape(N_FAST, R_F)
        wlc[N_FAST:, T : T + R_S] = dc[nf_rows:].reshape(N_SLOW, R_S)
        in_maps.append({"wl": wlc})

    import os

    trace_cores = None
    if os.environ.get("KERNEL_TRACE_ALL"):
        trace_cores = list(range(N_CORES))
    res = run_bass_kernel_spmd(
        nc,
        in_maps,
        core_ids=list(range(N_CORES)),
        trace=TRACE,
        trace_cores=trace_cores,
    )
    LAST_RESULTS = res

    out = np.empty((B, H), np.float32)
    for c in range(N_CORES):
        hd = res.results[c]["heads"]
        tl = res.results[c]["tails"]
        o = out[c * bpc : (c + 1) * bpc]
        o[:nf_rows, :T] = hd[:N_FAST, : R_F * T].reshape(nf_rows, T)
        o[nf_rows:, :T] = hd[N_FAST:, : R_S * T].reshape(N_SLOW * R_S, T)
        o[:nf_rows, T:] = tl[:N_FAST, : R_F * TL].reshape(nf_rows, TL)
        o[nf_rows:, T:] = tl[N_FAST:, : R_S * TL].reshape(N_SLOW * R_S, TL)
    return out


# revision 8
# speedup vs baseline: 1.0894x; 1.0669x over previous
"""Trainium2 Bass kernel for LyapunovSDELayer.

Reference computes, per batch element b with lam0 = current_lyapunov[b, 0]:
    path[b, 0] = lam0
    path[b, t] = clip(path[b, t-1] + KAPPA*(THETA - path[b, t-1]), 0, 1)

The step map is affine: lam -> 0.5*lam + 0.15, and for lam0 in [0, 1) the
iterates stay inside [0.15, 0.65] so the clip never binds.  Hence

    path[b, t] = THETA + 0.5**t * (lam0 - THETA)

0.5**t is a power of two so w_t * d is exact in fp32 and
fl(THETA + w_t*d) matches the reference fp32 scan to ~1 ulp; for
t >= 32 the value is exactly fl32(THETA) (the scan converges by t=26).

The kernel is a pure HBM-store-bandwidth problem (16 MB/core x 8 cores
vs ~2.9 TB/s chip HBM).  Structure:

  * the output is split into a `heads` region ([rows, 32], computed) and
    a `tails` region ([rows, 224], the constant fl32(THETA)); the host
    reassembles columns.  Tail stores read one small constant SBUF tile
    (read-only, reused by every store) with NO input dependency, so the
    stream starts right after the fixed NEFF preamble and has no
    write-after-read rotation stalls.
  * HWDGE descriptors are dealt to the 16 SDMA engines in blocks of 8
    by descriptor index, and engine 15 is reproducibly ~20% slower under
    load (21 vs 26 B/ns).  Partitions 120-127 (whose descriptors always
    land on engine 15) therefore carry 98 rows instead of 130; the
    balancing "extra" stores cover partitions 0:120 only, whose 120
    descriptors engine 15 never serves.  This equalizes engine finish
    times (~41 us) instead of letting engine 15 trail by ~10 us.
  * heads are computed in two whole-tile passes (DVE broadcast
    tensor_tensor for w_t*d, one ACT activation for +THETA) and stored
    via the Activation HWDGE queue so they interleave with the tail
    stream mid-flight.
  * emission order IS program order for Tile's dependency tracking: the
    input load is emitted before the tensor_tensor that reads it, and
    all constant-tile memsets are emitted before any store that reads
    the tile, so the only waits are genuine RAW edges.
  * all DRAM store regions are padded so per-partition runs never
    collapse into one contiguous block: a collapsed AP takes the slow
    8-engine "spray" path (~12 B/ns/engine vs 26.5 for strided stores).
"""

import sys
import types

import numpy as np

import concourse.bacc as bacc
import concourse.mybir as mybir
from concourse.tile import TileContext
from concourse.bass_utils import run_bass_kernel_spmd

# If BASS_TRACE is set in the environment, run_bass_kernel_spmd imports
# antenv.axon_hooks, which this image lacks — register a no-op stub so
# that path degrades to "no trace" instead of crashing.
try:
    import antenv.axon_hooks  # noqa: F401
except ImportError:
    try:
        import antenv

        _stub = types.ModuleType("antenv.axon_hooks")
        _stub.get_axon_ntff_profile_hook = lambda: None
        _stub.set_axon_ntff_profile_hook = lambda h: None
        sys.modules["antenv.axon_hooks"] = _stub
        antenv.axon_hooks = _stub
    except Exception:
        pass

THETA = 0.3
KAPPA = 0.5
N_CORES = 8
P = 128

# rows per partition: partitions 0..119 vs engine-15 partitions 120..127
R_F = 130
R_S = 98
N_SLOW = 8
N_FAST = P - N_SLOW
# uniform-row store schedule (all 128 partitions, R_S rows total) and
# extra-row schedule (partitions 0:N_FAST, R_F - R_S rows total)
UNIFORM_SCHED = [1, 2, 4, 5, 8, 16, 16, 16, 16, 14]
EXTRA_SCHED = [16, 16]
FILLS = [1, 4, 8, 16]  # progressive constant-tile fill boundaries (rows)
PAD = 16  # free-dim padding (elements) keeping DRAM APs partition-strided

_NC_CACHE = {}

# test harness hook: set by test.py to capture BassKernelResults
LAST_RESULTS = None
TRACE = False


def _build(bpc: int, H: int):
    T = min(32, H)
    TL = H - T
    f32 = mybir.dt.float32
    assert bpc == N_FAST * R_F + N_SLOW * R_S
    assert sum(UNIFORM_SCHED) == R_S and sum(EXTRA_SCHED) == R_F - R_S
    CG = FILLS[-1]
    assert max(UNIFORM_SCHED + EXTRA_SCHED) <= CG

    nc = bacc.Bacc()
    wl = nc.dram_tensor("wl", [P, T + R_F], f32, kind="ExternalInput")
    heads = nc.dram_tensor("heads", [P, R_F * T + PAD], f32, kind="ExternalOutput")
    tails = nc.dram_tensor("tails", [P, R_F * TL + PAD], f32, kind="ExternalOutput")

    with TileContext(nc) as tc:
        with tc.tile_pool(name="work", bufs=1) as pool:
            wl_sb = pool.tile([P, T + R_F], f32)
            ct = pool.tile([P, CG * TL], f32)
            prod = pool.tile([P, R_F * T], f32)
            ht = pool.tile([P, R_F * T], f32)

            # DVE: progressive constant-tile fill (all emitted before any
            # store reads the tile, so stores carry only RAW edges)
            f0 = 0
            for f1 in FILLS:
                nc.vector.memset(ct[:, f0 * TL : f1 * TL], THETA)
                f0 = f1

            # Dual-queue ramp: each DMA dispatch costs ~0.7 us of serial
            # sequencer time, so the input load goes out on the (otherwise
            # idle until ~14 us) ACT queue and the first tail stores
            # alternate SP/ACT — the engine rings fill twice as fast.
            # Later stores all ride SP so ACT is free for the head path.
            nc.scalar.dma_start(out=wl_sb, in_=wl[:, :])
            r0 = 0
            for i, g in enumerate(UNIFORM_SCHED):
                eng = nc.scalar if (i % 2 == 1 and i < 6) else nc.sync
                eng.dma_start(
                    out=tails[:, r0 * TL : (r0 + g) * TL], in_=ct[:, : g * TL]
                )
                r0 += g
            assert r0 == R_S
            for g in EXTRA_SCHED:
                nc.sync.dma_start(
                    out=tails[:N_FAST, r0 * TL : (r0 + g) * TL],
                    in_=ct[:N_FAST, : g * TL],
                )
                r0 += g
            assert r0 == R_F

            # DVE: head product prod = w_t * d (RAW on the input load)
            wt = wl_sb[:, :T]
            d = wl_sb[:, T : T + R_F]
            d3 = d.rearrange("p (r one) -> p r one", one=1).broadcast_to((P, R_F, T))
            w3 = wt.rearrange("p (one t) -> p one t", one=1).broadcast_to((P, R_F, T))
            p3 = prod.rearrange("p (r t) -> p r t", t=T)
            nc.vector.tensor_tensor(out=p3, in0=d3, in1=w3, op=mybir.AluOpType.mult)

            # ACT: +THETA pass, then the head stores on the ACT HWDGE
            # queue (engines round-robin between the two rings, so heads
            # interleave into the tail stream mid-flight)
            nc.scalar.activation(
                out=ht,
                in_=prod,
                func=mybir.ActivationFunctionType.Copy,
                bias=THETA,
                scale=1.0,
            )
            nc.scalar.dma_start(out=heads[:, : R_S * T], in_=ht[:, : R_S * T])
            nc.scalar.dma_start(
                out=heads[:N_FAST, R_S * T : R_F * T],
                in_=ht[:N_FAST, R_S * T : R_F * T],
            )
    nc.finalize()
    return nc


def kernel(current_lyapunov: np.ndarray, horizon) -> np.ndarray:
    global LAST_RESULTS
    lam0 = np.ascontiguousarray(np.asarray(current_lyapunov, np.float32)).reshape(-1)
    H = int(horizon)
    B = lam0.shape[0]
    assert B % N_CORES == 0
    bpc = B // N_CORES
    T = min(32, H)
    TL = H - T

    key = (bpc, H)
    if key not in _NC_CACHE:
        _NC_CACHE[key] = _build(bpc, H)
    nc = _NC_CACHE[key]

    # w_t = 0.5**t exact powers of two; d = lam0 - THETA (numpy fp32 sub
    # == device fp32 sub, bit-identical)
    w = (0.5 ** np.arange(T, dtype=np.float64)).astype(np.float32)
    d_host = (lam0 - np.float32(THETA)).astype(np.float32)
    nf_rows = N_FAST * R_F
    in_maps = []
    for c in range(N_CORES):
        dc = d_host[c * bpc : (c + 1) * bpc]
        wlc = np.zeros((P, T + R_F), np.float32)
        wlc[:, :T] = w
        wlc[:N_FAST, T : T + R_F] = dc[:nf_rows].reshape(N_FAST, R_F)
        wlc[N_FAST:, T : T + R_S] = dc[nf_rows:].reshape(N_SLOW, R_S)
        in_maps.append({"wl": wlc})

    import os

    trace_cores = None
    if os.environ.get("KERNEL_TRACE_ALL"):
        trace_cores = list(range(N_CORES))
    res = run_bass_kernel_spmd(
        nc,
        in_maps,
        core_ids=list(range(N_CORES)),
        trace=TRACE,
        trace_cores=trace_cores,
    )
    LAST_RESULTS = res

    out = np.empty((B, H), np.float32)
    for c in range(N_CORES):
        hd = res.results[c]["heads"]
        tl = res.results[c]["tails"]
        o = out[c * bpc : (c + 1) * bpc]
        o[:nf_rows, :T] = hd[:N_FAST, : R_F * T].reshape(nf_rows, T)
        o[nf_rows:, :T] = hd[N_FAST:, : R_S * T].reshape(N_SLOW * R_S, T)
        o[:nf_rows, T:] = tl[:N_FAST, : R_F * TL].reshape(nf_rows, TL)
        o[nf_rows:, T:] = tl[N_FAST:, : R_S * TL].reshape(N_SLOW * R_S, TL)
    return out


# revision 9
# speedup vs baseline: 1.0987x; 1.0086x over previous
"""Trainium2 Bass kernel for LyapunovSDELayer.

Reference computes, per batch element b with lam0 = current_lyapunov[b, 0]:
    path[b, 0] = lam0
    path[b, t] = clip(path[b, t-1] + KAPPA*(THETA - path[b, t-1]), 0, 1)

The step map is affine: lam -> 0.5*lam + 0.15, and for lam0 in [0, 1) the
iterates stay inside [0.15, 0.65] so the clip never binds.  Hence

    path[b, t] = THETA + 0.5**t * (lam0 - THETA)

0.5**t is a power of two so w_t * d is exact in fp32 and
fl(THETA + w_t*d) matches the reference fp32 scan to ~1 ulp; for
t >= 32 the value is exactly fl32(THETA) (the scan converges by t=26).

The kernel is a pure HBM-store-bandwidth problem (16 MB/core x 8 cores
vs ~2.9 TB/s chip HBM).  Structure:

  * the output is split into a `heads` region ([rows, 32], computed) and
    a `tails` region ([rows, 224], the constant fl32(THETA)); the host
    reassembles columns.  Tail stores read one small constant SBUF tile
    (read-only, reused by every store) with NO input dependency, so the
    stream starts right after the fixed NEFF preamble and has no
    write-after-read rotation stalls.
  * HWDGE descriptors are dealt to the 16 SDMA engines in blocks of 8
    by descriptor index, and engine 15 is reproducibly ~20% slower under
    load (21 vs 26 B/ns).  Partitions 120-127 (whose descriptors always
    land on engine 15) therefore carry 98 rows instead of 130; the
    balancing "extra" stores cover partitions 0:120 only, whose 120
    descriptors engine 15 never serves.  This equalizes engine finish
    times (~41 us) instead of letting engine 15 trail by ~10 us.
  * heads are computed in two whole-tile passes (DVE broadcast
    tensor_tensor for w_t*d, one ACT activation for +THETA) and stored
    via the Activation HWDGE queue so they interleave with the tail
    stream mid-flight.
  * emission order IS program order for Tile's dependency tracking: the
    input load is emitted before the tensor_tensor that reads it, and
    all constant-tile memsets are emitted before any store that reads
    the tile, so the only waits are genuine RAW edges.
  * all DRAM store regions are padded so per-partition runs never
    collapse into one contiguous block: a collapsed AP takes the slow
    8-engine "spray" path (~12 B/ns/engine vs 26.5 for strided stores).
"""

import sys
import types

import numpy as np

import concourse.bacc as bacc
import concourse.mybir as mybir
from concourse.tile import TileContext
from concourse.bass_utils import run_bass_kernel_spmd

# If BASS_TRACE is set in the environment, run_bass_kernel_spmd imports
# antenv.axon_hooks, which this image lacks — register a no-op stub so
# that path degrades to "no trace" instead of crashing.
try:
    import antenv.axon_hooks  # noqa: F401
except ImportError:
    try:
        import antenv

        _stub = types.ModuleType("antenv.axon_hooks")
        _stub.get_axon_ntff_profile_hook = lambda: None
        _stub.set_axon_ntff_profile_hook = lambda h: None
        sys.modules["antenv.axon_hooks"] = _stub
        antenv.axon_hooks = _stub
    except Exception:
        pass

THETA = 0.3
KAPPA = 0.5
N_CORES = 8
P = 128

# rows per partition: partitions 0..119 vs engine-15 partitions 120..127
R_F = 129
R_S = 113
N_SLOW = 8
N_FAST = P - N_SLOW
# uniform-row store schedule (all 128 partitions, R_S rows total) and
# extra-row schedule (partitions 0:N_FAST, R_F - R_S rows total)
UNIFORM_SCHED = [1, 2, 4, 5, 8, 16, 16, 16, 16, 16, 13]
EXTRA_SCHED = [16]
FILLS = [1, 4, 8, 16]  # progressive constant-tile fill boundaries (rows)
PAD = 16  # free-dim padding (elements) keeping DRAM APs partition-strided

_NC_CACHE = {}

# test harness hook: set by test.py to capture BassKernelResults
LAST_RESULTS = None
TRACE = False


def _build(bpc: int, H: int):
    T = min(32, H)
    TL = H - T
    f32 = mybir.dt.float32
    assert bpc == N_FAST * R_F + N_SLOW * R_S
    assert sum(UNIFORM_SCHED) == R_S and sum(EXTRA_SCHED) == R_F - R_S
    CG = FILLS[-1]
    assert max(UNIFORM_SCHED + EXTRA_SCHED) <= CG

    nc = bacc.Bacc()
    wl = nc.dram_tensor("wl", [P, T + R_F], f32, kind="ExternalInput")
    heads = nc.dram_tensor("heads", [P, R_F * T + PAD], f32, kind="ExternalOutput")
    tails = nc.dram_tensor("tails", [P, R_F * TL + PAD], f32, kind="ExternalOutput")

    with TileContext(nc) as tc:
        with tc.tile_pool(name="work", bufs=1) as pool:
            wl_sb = pool.tile([P, T + R_F], f32)
            ct = pool.tile([P, CG * TL], f32)
            prod = pool.tile([P, R_F * T], f32)
            ht = pool.tile([P, R_F * T], f32)

            # DVE: progressive constant-tile fill (all emitted before any
            # store reads the tile, so stores carry only RAW edges)
            f0 = 0
            for f1 in FILLS:
                nc.vector.memset(ct[:, f0 * TL : f1 * TL], THETA)
                f0 = f1

            # Dual-queue ramp: each DMA dispatch costs ~0.7 us of serial
            # sequencer time, so the input load goes out on the (otherwise
            # idle until ~14 us) ACT queue and the first tail stores
            # alternate SP/ACT — the engine rings fill twice as fast.
            # Later stores all ride SP so ACT is free for the head path.
            nc.scalar.dma_start(out=wl_sb, in_=wl[:, :])
            r0 = 0
            for i, g in enumerate(UNIFORM_SCHED):
                eng = nc.scalar if (i % 2 == 1 and i < 6) else nc.sync
                eng.dma_start(
                    out=tails[:, r0 * TL : (r0 + g) * TL], in_=ct[:, : g * TL]
                )
                r0 += g
            assert r0 == R_S
            for g in EXTRA_SCHED:
                nc.sync.dma_start(
                    out=tails[:N_FAST, r0 * TL : (r0 + g) * TL],
                    in_=ct[:N_FAST, : g * TL],
                )
                r0 += g
            assert r0 == R_F

            # DVE: head product prod = w_t * d (RAW on the input load)
            wt = wl_sb[:, :T]
            d = wl_sb[:, T : T + R_F]
            d3 = d.rearrange("p (r one) -> p r one", one=1).broadcast_to((P, R_F, T))
            w3 = wt.rearrange("p (one t) -> p one t", one=1).broadcast_to((P, R_F, T))
            p3 = prod.rearrange("p (r t) -> p r t", t=T)
            nc.vector.tensor_tensor(out=p3, in0=d3, in1=w3, op=mybir.AluOpType.mult)

            # ACT: +THETA pass, then the head stores on the ACT HWDGE
            # queue (engines round-robin between the two rings, so heads
            # interleave into the tail stream mid-flight)
            nc.scalar.activation(
                out=ht,
                in_=prod,
                func=mybir.ActivationFunctionType.Copy,
                bias=THETA,
                scale=1.0,
            )
            nc.scalar.dma_start(out=heads[:, : R_S * T], in_=ht[:, : R_S * T])
            nc.scalar.dma_start(
                out=heads[:N_FAST, R_S * T : R_F * T],
                in_=ht[:N_FAST, R_S * T : R_F * T],
            )
    nc.finalize()
    return nc


def kernel(current_lyapunov: np.ndarray, horizon) -> np.ndarray:
    global LAST_RESULTS
    lam0 = np.ascontiguousarray(np.asarray(current_lyapunov, np.float32)).reshape(-1)
    H = int(horizon)
    B = lam0.shape[0]
    assert B % N_CORES == 0
    bpc = B // N_CORES
    T = min(32, H)
    TL = H - T

    key = (bpc, H)
    if key not in _NC_CACHE:
        _NC_CACHE[key] = _build(bpc, H)
    nc = _NC_CACHE[key]

    # w_t = 0.5**t exact powers of two; d = lam0 - THETA (numpy fp32 sub
    # == device fp32 sub, bit-identical)
    w = (0.5 ** np.arange(T, dtype=np.float64)).astype(np.float32)
    d_host = (lam0 - np.float32(THETA)).astype(np.float32)
    nf_rows = N_FAST * R_F
    in_maps = []
    for c in range(N_CORES):
        dc = d_host[c * bpc : (c + 1) * bpc]
        wlc = np.zeros((P, T + R_F), np.float32)
        wlc[:, :T] = w
        wlc[:N_FAST, T : T + R_F] = dc[:nf_rows].reshape(N_FAST, R_F)
        wlc[N_FAST:, T : T + R_S] = dc[nf_rows:].reshape(N_SLOW, R_S)
        in_maps.append({"wl": wlc})

    import os

    trace_cores = None
    if os.environ.get("KERNEL_TRACE_ALL"):
        trace_cores = list(range(N_CORES))
    res = run_bass_kernel_spmd(
        nc,
        in_maps,
        core_ids=list(range(N_CORES)),
        trace=TRACE,
        trace_cores=trace_cores,
    )
    LAST_RESULTS = res

    out = np.empty((B, H), np.float32)
    for c in range(N_CORES):
        hd = res.results[c]["heads"]
        tl = res.results[c]["tails"]
        o = out[c * bpc : (c + 1) * bpc]
        o[:nf_rows, :T] = hd[:N_FAST, : R_F * T].reshape(nf_rows, T)
        o[nf_rows:, :T] = hd[N_FAST:, : R_S * T].reshape(N_SLOW * R_S, T)
        o[:nf_rows, T:] = tl[:N_FAST, : R_F * TL].reshape(nf_rows, TL)
        o[nf_rows:, T:] = tl[N_FAST:, : R_S * TL].reshape(N_SLOW * R_S, TL)
    return out
